# revision 6
# baseline (speedup 1.0000x reference)
"""CGC MoE routing kernel for Trainium2, 8-core data-parallel over batch.

Problem (per reference):
  B=4096, D_FULL=1024, D_T1=D_T2=512, experts: 4 shared (on x_full),
  4 task-1 (on x_task1), 4 task-2 (on x_task2); each expert is a 2-layer
  ReLU MLP (hidden 512, out 256). Three softmax gates combine expert
  outputs into (out_sh, out1, out2), each [4096, 256] fp32.

Strategy: shard the batch 8 ways (512 rows/core), replicate weights.
Each core computes all 12 experts + gates for its shard; host concats.

All matmul operands are cast to bf16 ON THE HOST and packed into the
exact SBUF layout; inputs are bundled into a handful of large HWDGE
DMAs (per-DMA fixed cost ~1.5-2us, so few big transfers beat many
small ones) ordered by first use:
  sync ring:   x1 | w1_t1_{0,1,2} | w1_t1_3+xf+x2 | w1_sh | w1_t2
  scalar ring: consts(gw,gb,b1,b2) | w2_t1 | w2_sh | w2_t2
Layers:
  L1: H[h,b]  = sum_d W1[d,h].T @ X'[d,b]   (per-hi PSUM bank, ACT ReLU
      with per-partition b1 bias)
  L2: EO[b,o] = sum_h H[h,b].T  @ W2[h,o] accumulated into one
      [128, 4*256] PSUM tile; one ones-row K=1 matmul adds b2 for all
      4 b-subtiles; ONE batched ACT ReLU -> bf16 eo (the last expert
      instead splits eo per b-subtile so its combine pipeline drains
      early).
  Gates: logits[g,b] on PE; exp via ACT with bias; 4 PE-transposes into
      one PSUM tile; batched copy + rowsum/recip/scale -> pre-normalized
      gate columns. Gate sets are emitted interleaved between t1
      experts (g1 after t1_1, gsh after t1_2, g2 after t1_3).
  Combine: acc[b,o] (+)= EO_e[b,o] * gate_col[b,1] on DVE
      (scalar_tensor_tensor); acc is one [128, 4*256] fp32 tile per
      output; one output DMA per output (split in halves for the two
      last-finishing outputs); host unpacks [128,4,256].
"""
import sys
import numpy as np

sys.path.insert(0, "/opt/trn_rl_repo")

import ml_dtypes

import concourse.bass as bass
import concourse.mybir as mybir
import concourse.tile as tile
import concourse.masks as masks
from concourse.bass_utils import run_bass_kernel_spmd

F32 = mybir.dt.float32
BF16 = mybir.dt.bfloat16
BF = ml_dtypes.bfloat16

B = 4096
N_CORES = 8
BC = B // N_CORES          # 512 rows per core
DF, D1, D2 = 1024, 512, 512
HID, OUT = 512, 256
NB = BC // 128             # 4 batch tiles per core
NH = HID // 128            # 4 hidden tiles
E = 4                      # experts per group
GROUPS = ["t1", "sh", "t2"]           # processing order
GND = {"t1": D1 // 128, "sh": DF // 128, "t2": D2 // 128}
# gate sets: 0=gsh(x_full,12), 1=g1(x_task1,8), 2=g2(x_task2,8)
GATE_NG = [12, 8, 8]
GATE_X = ["sh", "t1", "t2"]           # which x feeds each gate
GATE_WOFF = [0, 96, 128]              # col offset in packed gate weights

# ---- bundled DMA layout (all bf16, [128, width] each) ----------------------
# widths in bf16 elements per partition
W_X = {g: GND[g] * BC for g in GROUPS}
W_W1 = {g: GND[g] * HID for g in GROUPS}
W_W2 = E * NH * OUT
BUNDLES = [
    ("bun_x1", [("x_t1", W_X["t1"])]),
    ("bun_w1a", [("w1_t1_0", W_W1["t1"]), ("w1_t1_1", W_W1["t1"]),
                 ("w1_t1_2", W_W1["t1"])]),
    ("bun_w1b", [("w1_t1_3", W_W1["t1"]), ("x_sh", W_X["sh"]),
                 ("x_t2", W_X["t2"])]),
    ("bun_w1sh", [(f"w1_sh_{e}", W_W1["sh"]) for e in range(E)]),
    ("bun_w1t2", [(f"w1_t2_{e}", W_W1["t2"]) for e in range(E)]),
]
SCALAR_BUNDLES = [
    ("bun_w2t1", [("w2_t1", W_W2)]),
    ("bun_w2sh", [("w2_sh", W_W2)]),
    ("bun_w2t2", [("w2_t2", W_W2)]),
]


def _gidx(group):
    return GROUPS.index(group)


def _legalize_waits(nc, max_waits: int = 1):
    """This walrus build supports a single sync wait per instruction;
    hoist extra waits onto standalone single-wait EventSemaphore
    instructions inserted just before (same engine, same order)."""
    uid = 0
    for f in nc.m.functions:
        for blk in f.blocks:
            out = []
            changed = False
            for inst in blk.instructions:
                si = inst.sync_info
                ow = list(si.on_wait) if si and si.on_wait else []
                if len(ow) > max_waits:
                    changed = True
                    for w in ow[:-max_waits]:
                        ev = mybir.InstEventSemaphore(
                            name=f"legalw-{uid}",
                            sync_info=mybir.SyncInfo(on_wait=[w], on_update=[]),
                        )
                        uid += 1
                        ev.engine = inst.engine
                        out.append(ev)
                    inst.sync_info = mybir.SyncInfo(
                        on_wait=ow[-max_waits:],
                        on_update=list(si.on_update) if si.on_update else [],
                    )
                out.append(inst)
            if changed:
                blk.instructions = out
    return nc


def _build_nc():
    nc = bass.Bass()

    def din(name, shape, dt):
        return nc.declare_dram_parameter(name, list(shape), dt, isOutput=False)

    bun_in = {}
    for bname, parts in BUNDLES + SCALAR_BUNDLES:
        w = sum(p[1] for p in parts)
        bun_in[bname] = din(bname, (128, w), BF16)
    b1in = din("b1all", (128, 12 * NH), F32)        # [:, k*NH+hi]
    b2in = din("b2all", (1, 12 * NB * OUT), BF16)   # b2 repeated per btile
    gwin = din("gwall", (128, 160), BF16)
    gbin = din("gball", (12, 3), F32)

    outs = [nc.declare_dram_parameter(n, [128, NB * OUT], F32, isOutput=True)
            for n in ("o_sh", "o_t1", "o_t2")]

    with tile.TileContext(nc) as tc:
        _emit(nc, tc, bun_in, b1in, b2in, gwin, gbin, outs)
    _legalize_waits(nc)
    return nc


def _emit(nc, tc, bun_in, b1in, b2in, gwin, gbin, outs):
    from contextlib import ExitStack
    ctx = ExitStack()
    with ctx:
        wp = ctx.enter_context(tc.tile_pool(name="wp", bufs=1))
        cp = ctx.enter_context(tc.tile_pool(name="cp", bufs=1))
        hp = ctx.enter_context(tc.tile_pool(name="hp", bufs=2))
        eop = ctx.enter_context(tc.tile_pool(name="eop", bufs=6))
        accp = ctx.enter_context(tc.tile_pool(name="accp", bufs=1))
        gp = ctx.enter_context(tc.tile_pool(name="gp", bufs=1))
        ps1 = ctx.enter_context(tc.tile_pool(name="ps1", bufs=4, space="PSUM"))
        ps2 = ctx.enter_context(tc.tile_pool(name="ps2", bufs=2, space="PSUM"))

        # ---- input DMAs (HWDGE), bundled, ordered by first use ----------
        # seg maps logical key -> (tile, col offset, width)
        seg = {}

        def load_bundle(bname, parts, engine):
            w = sum(p[1] for p in parts)
            t = wp.tile([128, w], BF16, name=bname, tag=bname)
            engine.dma_start(t[:], bun_in[bname][:])
            off = 0
            for key, width in parts:
                seg[key] = (t, off, width)
                off += width

        # interleave: consts first on scalar ring (tiny), then the big
        # sync-ring stream; W2 groups stream on the scalar ring.
        load_bundle(*BUNDLES[0], nc.sync)           # x1
        gwt = cp.tile([128, 160], BF16, tag="gw")
        nc.scalar.dma_start(gwt[:], gwin[:])
        gbt = cp.tile([12, 3], F32, tag="gb")
        nc.scalar.dma_start(gbt[:], gbin[:])
        b1t = cp.tile([128, 12 * NH], F32, tag="b1")
        nc.scalar.dma_start(b1t[:], b1in[:])
        b2t = cp.tile([1, 12 * NB * OUT], BF16, tag="b2")
        nc.scalar.dma_start(b2t[:], b2in[:])
        for bname, parts in BUNDLES[1:]:
            load_bundle(bname, parts, nc.sync)
        for bname, parts in SCALAR_BUNDLES:
            load_bundle(bname, parts, nc.scalar)

        def xs(g, di):
            t, off, _ = seg[f"x_{g}"]
            return t[:, off + di * BC: off + (di + 1) * BC]

        def w1s(g, e, di, hi):
            t, off, _ = seg[f"w1_{g}_{e}"]
            o = off + di * HID + hi * 128
            return t[:, o: o + 128]

        def w2s(g, e, hi):
            t, off, _ = seg[f"w2_{g}"]
            o = off + (e * NH + hi) * OUT
            return t[:, o: o + OUT]

        # identity for PE transpose; ones row for K=1 bias matmuls
        # (memset/iota are invalid ISA at bf16 -> build fp32, cast-copy)
        ident32 = cp.tile([128, 128], F32, tag="ident32")
        masks.make_identity(nc, ident32[:])
        ident = cp.tile([128, 128], BF16, tag="ident")
        nc.vector.tensor_copy(ident[:], ident32[:])
        ones32 = cp.tile([1, 128], F32, tag="ones32")
        nc.vector.memset(ones32[:], 1.0)
        ones = cp.tile([1, 128], BF16, tag="ones")
        nc.vector.tensor_copy(ones[:], ones32[:])

        # accumulator tiles [128, NB*OUT] fp32 per output
        acc = [accp.tile([128, NB * OUT], F32, name=f"acc{o}", tag=f"acc{o}")
               for o in range(3)]
        acc_init = [[False] * NB for _ in range(3)]

        # ---- expert bodies ---------------------------------------------
        def emit_l1(g, e, d_outer=False):
            nd = GND[g]
            k = _gidx(g) * E + e
            h = hp.tile([128, NH * BC], BF16, name="h", tag="h")

            def bias(hi):
                return b1t[:, k * NH + hi: k * NH + hi + 1]

            if d_outer:
                ps = [ps1.tile([128, BC], F32, name=f"p1_{hi}", tag="p1")
                      for hi in range(NH)]
                for di in range(nd):
                    for hi in range(NH):
                        nc.tensor.matmul(
                            ps[hi][:], w1s(g, e, di, hi), xs(g, di),
                            start=(di == 0), stop=(di == nd - 1))
                for hi in range(NH):
                    nc.scalar.activation(h[:, hi * BC:(hi + 1) * BC],
                                         ps[hi][:],
                                         mybir.ActivationFunctionType.Relu,
                                         bias=bias(hi))
            else:
                for hi in range(NH):
                    p1 = ps1.tile([128, BC], F32, name="p1", tag="p1")
                    for di in range(nd):
                        nc.tensor.matmul(
                            p1[:], w1s(g, e, di, hi), xs(g, di),
                            start=(di == 0), stop=(di == nd - 1))
                    nc.scalar.activation(h[:, hi * BC:(hi + 1) * BC], p1[:],
                                         mybir.ActivationFunctionType.Relu,
                                         bias=bias(hi))
            return h

        def emit_l2(g, e, h, split_eo=False):
            k = _gidx(g) * E + e
            p2 = ps2.tile([128, NB * OUT], F32, name="p2", tag="p2")
            for bi in range(NB):
                dst = p2[:, bi * OUT:(bi + 1) * OUT]
                for hi in range(NH):
                    nc.tensor.matmul(
                        dst,
                        h[:, hi * BC + bi * 128: hi * BC + (bi + 1) * 128],
                        w2s(g, e, hi),
                        start=(hi == 0), stop=False)
                # K=1 ones matmul adds b2 and closes this subtile's group
                nc.tensor.matmul(
                    dst, ones[:],
                    b2t[0:1, (k * NB + bi) * OUT:(k * NB + bi + 1) * OUT],
                    start=False, stop=True)
            eo = eop.tile([128, NB * OUT], BF16, name="eo", tag="eo")
            if split_eo:
                for bi in range(NB):
                    nc.scalar.activation(eo[:, bi * OUT:(bi + 1) * OUT],
                                         p2[:, bi * OUT:(bi + 1) * OUT],
                                         mybir.ActivationFunctionType.Relu)
            else:
                nc.scalar.activation(eo[:], p2[:],
                                     mybir.ActivationFunctionType.Relu)
            return eo

        # expert -> (output index, gate set, gate column) contributions
        # gates: gsh over [t1(0-3), t2(4-7), sh(8-11)]
        #        g1  over [t1(0-3), sh(4-7)]; g2 over [t2(0-3), sh(4-7)]
        def contributions(g, e):
            if g == "t1":
                return [(0, 0, e), (1, 1, e)]
            elif g == "t2":
                return [(0, 0, 4 + e), (2, 2, e)]
            else:
                return [(0, 0, 8 + e), (1, 1, 4 + e), (2, 2, 4 + e)]

        # processing order: t1(0..3), sh(0..3), t2(0..3)
        def _is_last_contrib(g, e, o):
            if o == 1:
                return g == "sh" and e == E - 1
            return g == "t2" and e == E - 1

        gate_ct = [None, None, None]

        def emit_combine(g, e, eo, split_dma=False):
            for bi in range(NB):
                eo_s = eo[:, bi * OUT:(bi + 1) * OUT]
                for (o, gs, col) in contributions(g, e):
                    ng = GATE_NG[gs]
                    g_ap = gate_ct[gs][:, bi * ng + col: bi * ng + col + 1]
                    a = acc[o][:, bi * OUT:(bi + 1) * OUT]
                    if not acc_init[o][bi]:
                        nc.vector.tensor_scalar_mul(a, eo_s, g_ap)
                        acc_init[o][bi] = True
                    else:
                        nc.vector.scalar_tensor_tensor(
                            a, eo_s, g_ap, a,
                            op0=mybir.AluOpType.mult,
                            op1=mybir.AluOpType.add)
                if split_dma and bi == 1:
                    for (o, gs, col) in contributions(g, e):
                        if _is_last_contrib(g, e, o):
                            half = 2 * OUT
                            nc.sync.dma_start(outs[o][:, :half],
                                              acc[o][:, :half])
            for (o, gs, col) in contributions(g, e):
                if _is_last_contrib(g, e, o):
                    if split_dma:
                        half = 2 * OUT
                        nc.sync.dma_start(outs[o][:, half:],
                                          acc[o][:, half:])
                    else:
                        nc.sync.dma_start(outs[o][:], acc[o][:])

        # ---- gates ------------------------------------------------------
        def emit_gate(gi):
            ng = GATE_NG[gi]
            gx = GATE_X[gi]
            nd = GND[gx]
            off = GATE_WOFF[gi]
            lg = ps1.tile([ng, BC], F32, name="lg", tag="p1")
            for di in range(nd):
                nc.tensor.matmul(
                    lg[:], gwt[:, off + di * ng: off + (di + 1) * ng],
                    xs(gx, di),
                    start=(di == 0), stop=(di == nd - 1))
            eg = gp.tile([ng, BC], BF16, name=f"eg{gi}", tag=f"eg{gi}")
            nc.scalar.activation(eg[:], lg[:],
                                 mybir.ActivationFunctionType.Exp,
                                 bias=gbt[0:ng, gi:gi + 1])
            pt = ps1.tile([128, NB * ng], BF16, name="gtr", tag="p1")
            for bi in range(NB):
                nc.tensor.transpose(pt[:, bi * ng:(bi + 1) * ng],
                                    eg[:, bi * 128:(bi + 1) * 128],
                                    ident[:ng, :ng])
            ct = gp.tile([128, NB * ng], F32, name=f"ct{gi}", tag=f"ct{gi}")
            nc.vector.tensor_copy(ct[:], pt[:])
            st = gp.tile([128, NB], F32, name=f"st{gi}", tag=f"st{gi}")
            rt = gp.tile([128, NB], F32, name=f"rt{gi}", tag=f"rt{gi}")
            for bi in range(NB):
                nc.vector.tensor_reduce(st[:, bi:bi + 1],
                                        ct[:, bi * ng:(bi + 1) * ng],
                                        axis=mybir.AxisListType.X,
                                        op=mybir.AluOpType.add)
            nc.vector.reciprocal(rt[:], st[:])
            for bi in range(NB):
                # pre-scale: combine uses normalized gates directly
                nc.vector.tensor_scalar_mul(ct[:, bi * ng:(bi + 1) * ng],
                                            ct[:, bi * ng:(bi + 1) * ng],
                                            rt[:, bi:bi + 1])
            gate_ct[gi] = ct

        # ---- emission order --------------------------------------------
        # t1_0 (d-outer, earliest PE start), t1_1, g1, t1_2, gsh, t1_3,
        # g2, deferred t1 combines, then sh/t2 inline.
        eos = {}
        eos[("t1", 0)] = emit_l2("t1", 0, emit_l1("t1", 0, d_outer=True))
        eos[("t1", 1)] = emit_l2("t1", 1, emit_l1("t1", 1))
        emit_gate(1)
        eos[("t1", 2)] = emit_l2("t1", 2, emit_l1("t1", 2))
        emit_gate(0)
        eos[("t1", 3)] = emit_l2("t1", 3, emit_l1("t1", 3))
        emit_gate(2)
        for e in range(E):
            emit_combine("t1", e, eos.pop(("t1", e)))
        for g in ("sh", "t2"):
            for e in range(E):
                last = (g == "t2" and e == E - 1)
                h = emit_l1(g, e)
                eo = emit_l2(g, e, h, split_eo=last)
                emit_combine(g, e, eo, split_dma=last)


_NC_CACHE = None


def _pack_shared(inputs):
    """Host-side packing into SBUF partition layouts + bf16 cast."""
    def pack_w1(w):   # [E, D, HID] -> per-e [128, nd*HID] fp32
        e_, d_, h_ = w.shape
        nd = d_ // 128
        wp = np.asarray(w, np.float32).reshape(e_, nd, 128, h_) \
            .transpose(0, 2, 1, 3).reshape(e_, 128, nd * h_)
        return [wp[e] for e in range(e_)]

    def pack_w2(w):   # [E, HID, OUT] -> [128, E*NH*OUT] fp32
        e_, h_, o_ = w.shape
        nh = h_ // 128
        return np.asarray(w, np.float32).reshape(e_, nh, 128, o_) \
            .transpose(2, 0, 1, 3).reshape(128, e_ * nh * o_)

    def pack_b1(b):   # [E, HID] -> [E, 128, NH]
        e_, h_ = b.shape
        nh = h_ // 128
        return np.asarray(b, np.float32).reshape(e_, nh, 128).transpose(0, 2, 1)

    def pack_gw(w):   # [D, ng] -> [128, nd*ng]
        d_, ng = w.shape
        nd = d_ // 128
        return np.asarray(w, np.float32).reshape(nd, 128, ng) \
            .transpose(1, 0, 2).reshape(128, nd * ng)

    W1 = {"t1": inputs["t1_W1"], "sh": inputs["sh_W1"], "t2": inputs["t2_W1"]}
    W2 = {"t1": inputs["t1_W2"], "sh": inputs["sh_W2"], "t2": inputs["t2_W2"]}
    B1 = {"t1": inputs["t1_b1"], "sh": inputs["sh_b1"], "t2": inputs["t2_b1"]}
    B2 = {"t1": inputs["t1_b2"], "sh": inputs["sh_b2"], "t2": inputs["t2_b2"]}

    # segment content (fp32) by key; x segments filled per core later
    segs = {}
    for g in GROUPS:
        for e, w in enumerate(pack_w1(W1[g])):
            segs[f"w1_{g}_{e}"] = w
        segs[f"w2_{g}"] = pack_w2(W2[g])

    m = {}
    b1 = np.stack([pack_b1(B1[g]) for g in GROUPS])       # [3, E, 128, NH]
    m["b1all"] = np.ascontiguousarray(
        b1.reshape(12, 128, NH).transpose(1, 0, 2).reshape(128, 12 * NH),
        dtype=np.float32)
    b2 = np.stack([np.asarray(B2[g], np.float32) for g in GROUPS])  # [3,E,OUT]
    b2r = np.broadcast_to(b2.reshape(12, 1, OUT), (12, NB, OUT))
    m["b2all"] = np.ascontiguousarray(b2r.reshape(1, 12 * NB * OUT)).astype(BF)

    gw = np.concatenate([pack_gw(inputs["gsh_W"]), pack_gw(inputs["g1_W"]),
                         pack_gw(inputs["g2_W"])], axis=1)  # [128, 160]
    m["gwall"] = np.ascontiguousarray(gw).astype(BF)
    gb = np.zeros((12, 3), np.float32)
    gb[:12, 0] = np.asarray(inputs["gsh_b"], np.float32)
    gb[:8, 1] = np.asarray(inputs["g1_b"], np.float32)
    gb[:8, 2] = np.asarray(inputs["g2_b"], np.float32)
    m["gball"] = gb
    return m, segs


def _pack_xT(x):      # [BC, D] -> [128, nd*BC] fp32
    bc, d_ = x.shape
    nd = d_ // 128
    xt = np.asarray(x, np.float32).T.reshape(nd, 128, bc) \
        .transpose(1, 0, 2).reshape(128, nd * bc)
    return xt


def _build_in_maps(inputs):
    m_const, segs = _pack_shared(inputs)
    xs = {"t1": inputs["x_task1"], "sh": inputs["x_full"],
          "t2": inputs["x_task2"]}

    # bundles without x segments are shared across cores
    shared_bundles = {}
    for bname, parts in BUNDLES + SCALAR_BUNDLES:
        if any(k.startswith("x_") for k, _ in parts):
            continue
        shared_bundles[bname] = np.ascontiguousarray(
            np.concatenate([segs[k] for k, _ in parts], axis=1)).astype(BF)

    in_maps = []
    for c in range(N_CORES):
        rows = slice(c * BC, (c + 1) * BC)
        m = dict(m_const)
        m.update(shared_bundles)
        xseg = {f"x_{g}": _pack_xT(np.asarray(xs[g])[rows]) for g in GROUPS}
        for bname, parts in BUNDLES:
            if bname in shared_bundles:
                continue
            m[bname] = np.ascontiguousarray(np.concatenate(
                [xseg[k] if k.startswith("x_") else segs[k] for k, _ in parts],
                axis=1)).astype(BF)
        in_maps.append(m)
    return in_maps


def _unpack_out(a):   # [128, NB*OUT] -> [BC, OUT]
    return np.ascontiguousarray(
        a.reshape(128, NB, OUT).transpose(1, 0, 2).reshape(BC, OUT))


def kernel(**inputs):
    global _NC_CACHE
    if _NC_CACHE is None:
        _NC_CACHE = _build_nc()
    nc = _NC_CACHE

    in_maps = _build_in_maps(inputs)
    res = run_bass_kernel_spmd(nc, in_maps, list(range(N_CORES)))
    full = []
    for name in ("o_sh", "o_t1", "o_t2"):
        full.append(np.concatenate(
            [_unpack_out(res.results[c][name]) for c in range(N_CORES)]))
    return tuple(full)


# revision 7
# speedup vs baseline: 1.1728x; 1.1728x over previous
"""CGC MoE routing kernel for Trainium2, 8-core data-parallel over batch.

Problem (per reference):
  B=4096, D_FULL=1024, D_T1=D_T2=512, experts: 4 shared (on x_full),
  4 task-1 (on x_task1), 4 task-2 (on x_task2); each expert is a 2-layer
  ReLU MLP (hidden 512, out 256). Three softmax gates combine expert
  outputs into (out_sh, out1, out2), each [4096, 256] fp32.

Strategy: shard the batch 8 ways (512 rows/core), replicate weights.
Each core computes all 12 experts + gates for its shard; host concats.

All matmul operands are cast to bf16 ON THE HOST and packed into the
exact SBUF layout; inputs are bundled into a handful of large HWDGE
DMAs (per-DMA fixed cost ~1.5-2us, so few big transfers beat many
small ones) ordered by first use:
  sync ring:   x1 | w1_t1_{0,1,2} | w1_t1_3+xf+x2 | w1_sh | w1_t2
  scalar ring: consts(gw,gb,b1,b2) | w2_t1 | w2_sh | w2_t2
Layers:
  L1: H[h,b]  = sum_d W1[d,h].T @ X'[d,b]   (per-hi PSUM bank, ACT ReLU
      with per-partition b1 bias)
  L2: EO[b,o] = sum_h H[h,b].T  @ W2[h,o] accumulated into one
      [128, 4*256] PSUM tile; one ones-row K=1 matmul adds b2 for all
      4 b-subtiles; ONE batched ACT ReLU -> bf16 eo (the last expert
      instead splits eo per b-subtile so its combine pipeline drains
      early).
  Gates: logits[g,b] on PE; exp via ACT with bias; 4 PE-transposes into
      one PSUM tile; batched copy + rowsum/recip/scale -> pre-normalized
      gate columns. Gate sets are emitted interleaved between t1
      experts (g1 after t1_1, gsh after t1_2, g2 after t1_3).
  Combine: acc[b,o] (+)= EO_e[b,o] * gate_col[b,1] on DVE
      (scalar_tensor_tensor); acc is one [128, 4*256] fp32 tile per
      output; one output DMA per output (split in halves for the two
      last-finishing outputs); host unpacks [128,4,256].
"""
import sys
import numpy as np

sys.path.insert(0, "/opt/trn_rl_repo")

import ml_dtypes

import concourse.bass as bass
import concourse.mybir as mybir
import concourse.tile as tile
import concourse.masks as masks
from concourse.bass_utils import run_bass_kernel_spmd

F32 = mybir.dt.float32
BF16 = mybir.dt.bfloat16
BF = ml_dtypes.bfloat16

B = 4096
N_CORES = 8
BC = B // N_CORES          # 512 rows per core
DF, D1, D2 = 1024, 512, 512
HID, OUT = 512, 256
NB = BC // 128             # 4 batch tiles per core
NH = HID // 128            # 4 hidden tiles
E = 4                      # experts per group
GROUPS = ["t1", "sh", "t2"]           # processing order
GND = {"t1": D1 // 128, "sh": DF // 128, "t2": D2 // 128}
# gate sets: 0=gsh(x_full,12), 1=g1(x_task1,8), 2=g2(x_task2,8)
GATE_NG = [12, 8, 8]
GATE_X = ["sh", "t1", "t2"]           # which x feeds each gate
GATE_WOFF = [0, 96, 128]              # col offset in packed gate weights

# ---- bundled DMA layout (all bf16, [128, width] each) ----------------------
# widths in bf16 elements per partition
W_X = {g: GND[g] * BC for g in GROUPS}
W_W1 = {g: GND[g] * HID for g in GROUPS}
W_W2 = E * NH * OUT
BUNDLES = [
    ("bun_x1", [("x_t1", W_X["t1"])]),
    ("bun_w10", [("w1_t1_0", W_W1["t1"])]),
    ("bun_w1a", [("w1_t1_1", W_W1["t1"]), ("w1_t1_2", W_W1["t1"])]),
    ("bun_w1b", [("w1_t1_3", W_W1["t1"]), ("x_sh", W_X["sh"]),
                 ("x_t2", W_X["t2"])]),
    ("bun_w1sh", [(f"w1_sh_{e}", W_W1["sh"]) for e in range(E)]),
    ("bun_w1t2", [(f"w1_t2_{e}", W_W1["t2"]) for e in range(E)]),
]
SCALAR_BUNDLES = [
    ("bun_w2t1", [("w2_t1", W_W2)]),
    ("bun_w2sh", [("w2_sh", W_W2)]),
    ("bun_w2t2", [("w2_t2", W_W2)]),
]


def _gidx(group):
    return GROUPS.index(group)


def _legalize_waits(nc, max_waits: int = 1):
    """This walrus build supports a single sync wait per instruction;
    hoist extra waits onto standalone single-wait EventSemaphore
    instructions inserted just before (same engine, same order)."""
    uid = 0
    for f in nc.m.functions:
        for blk in f.blocks:
            out = []
            changed = False
            for inst in blk.instructions:
                si = inst.sync_info
                ow = list(si.on_wait) if si and si.on_wait else []
                if len(ow) > max_waits:
                    changed = True
                    for w in ow[:-max_waits]:
                        ev = mybir.InstEventSemaphore(
                            name=f"legalw-{uid}",
                            sync_info=mybir.SyncInfo(on_wait=[w], on_update=[]),
                        )
                        uid += 1
                        ev.engine = inst.engine
                        out.append(ev)
                    inst.sync_info = mybir.SyncInfo(
                        on_wait=ow[-max_waits:],
                        on_update=list(si.on_update) if si.on_update else [],
                    )
                out.append(inst)
            if changed:
                blk.instructions = out
    return nc


def _build_nc():
    nc = bass.Bass()

    def din(name, shape, dt):
        return nc.declare_dram_parameter(name, list(shape), dt, isOutput=False)

    bun_in = {}
    for bname, parts in BUNDLES + SCALAR_BUNDLES:
        w = sum(p[1] for p in parts)
        bun_in[bname] = din(bname, (128, w), BF16)
    b1in = din("b1all", (128, 12 * NH), F32)        # [:, k*NH+hi]
    b2in = din("b2all", (1, 12 * NB * OUT), BF16)   # b2 repeated per btile
    gwin = din("gwall", (128, 160), BF16)
    gbin = din("gball", (12, 3), F32)

    outs = [nc.declare_dram_parameter(n, [128, NB * OUT], F32, isOutput=True)
            for n in ("o_sh", "o_t1", "o_t2")]

    with tile.TileContext(nc) as tc:
        _emit(nc, tc, bun_in, b1in, b2in, gwin, gbin, outs)
    _legalize_waits(nc)
    return nc


def _emit(nc, tc, bun_in, b1in, b2in, gwin, gbin, outs):
    from contextlib import ExitStack
    ctx = ExitStack()
    with ctx:
        wp = ctx.enter_context(tc.tile_pool(name="wp", bufs=1))
        cp = ctx.enter_context(tc.tile_pool(name="cp", bufs=1))
        hp = ctx.enter_context(tc.tile_pool(name="hp", bufs=2))
        eop = ctx.enter_context(tc.tile_pool(name="eop", bufs=6))
        accp = ctx.enter_context(tc.tile_pool(name="accp", bufs=1))
        gp = ctx.enter_context(tc.tile_pool(name="gp", bufs=1))
        ps1 = ctx.enter_context(tc.tile_pool(name="ps1", bufs=4, space="PSUM"))
        ps2 = ctx.enter_context(tc.tile_pool(name="ps2", bufs=2, space="PSUM"))

        # ---- input DMAs (HWDGE), bundled, ordered by first use ----------
        # seg maps logical key -> (tile, col offset, width)
        seg = {}

        def load_bundle(bname, parts, engine):
            w = sum(p[1] for p in parts)
            t = wp.tile([128, w], BF16, name=bname, tag=bname)
            engine.dma_start(t[:], bun_in[bname][:])
            off = 0
            for key, width in parts:
                seg[key] = (t, off, width)
                off += width

        # interleave: consts first on scalar ring (tiny), then the big
        # sync-ring stream; W2 groups stream on the scalar ring.
        load_bundle(*BUNDLES[0], nc.sync)           # x1
        load_bundle(*SCALAR_BUNDLES[0], nc.scalar)  # w2_t1 (needed ~12us)
        gwt = cp.tile([128, 160], BF16, tag="gw")
        nc.scalar.dma_start(gwt[:], gwin[:])
        gbt = cp.tile([12, 3], F32, tag="gb")
        nc.scalar.dma_start(gbt[:], gbin[:])
        b1t = cp.tile([128, 12 * NH], F32, tag="b1")
        nc.scalar.dma_start(b1t[:], b1in[:])
        b2t = cp.tile([1, 12 * NB * OUT], BF16, tag="b2")
        nc.scalar.dma_start(b2t[:], b2in[:])
        for bname, parts in BUNDLES[1:]:
            load_bundle(bname, parts, nc.sync)
        for bname, parts in SCALAR_BUNDLES[1:]:
            load_bundle(bname, parts, nc.scalar)

        def xs(g, di):
            t, off, _ = seg[f"x_{g}"]
            return t[:, off + di * BC: off + (di + 1) * BC]

        def w1s(g, e, di, hi):
            t, off, _ = seg[f"w1_{g}_{e}"]
            o = off + di * HID + hi * 128
            return t[:, o: o + 128]

        def w2s(g, e, hi):
            t, off, _ = seg[f"w2_{g}"]
            o = off + (e * NH + hi) * OUT
            return t[:, o: o + OUT]

        # identity for PE transpose; ones row for K=1 bias matmuls
        # (memset/iota are invalid ISA at bf16 -> build fp32, cast-copy)
        ident32 = cp.tile([128, 128], F32, tag="ident32")
        masks.make_identity(nc, ident32[:])
        ident = cp.tile([128, 128], BF16, tag="ident")
        nc.vector.tensor_copy(ident[:], ident32[:])
        ones32 = cp.tile([1, 128], F32, tag="ones32")
        nc.vector.memset(ones32[:], 1.0)
        ones = cp.tile([1, 128], BF16, tag="ones")
        nc.vector.tensor_copy(ones[:], ones32[:])

        # accumulator tiles [128, NB*OUT] fp32 per output
        acc = [accp.tile([128, NB * OUT], F32, name=f"acc{o}", tag=f"acc{o}")
               for o in range(3)]
        acc_init = [[False] * NB for _ in range(3)]

        # ---- expert bodies ---------------------------------------------
        def emit_l1(g, e, d_outer=False):
            nd = GND[g]
            k = _gidx(g) * E + e
            h = hp.tile([128, NH * BC], BF16, name="h", tag="h")

            def bias(hi):
                return b1t[:, k * NH + hi: k * NH + hi + 1]

            if d_outer:
                ps = [ps1.tile([128, BC], F32, name=f"p1_{hi}", tag="p1")
                      for hi in range(NH)]
                for di in range(nd):
                    for hi in range(NH):
                        nc.tensor.matmul(
                            ps[hi][:], w1s(g, e, di, hi), xs(g, di),
                            start=(di == 0), stop=(di == nd - 1))
                for hi in range(NH):
                    nc.scalar.activation(h[:, hi * BC:(hi + 1) * BC],
                                         ps[hi][:],
                                         mybir.ActivationFunctionType.Relu,
                                         bias=bias(hi))
            else:
                for hi in range(NH):
                    p1 = ps1.tile([128, BC], F32, name="p1", tag="p1")
                    for di in range(nd):
                        nc.tensor.matmul(
                            p1[:], w1s(g, e, di, hi), xs(g, di),
                            start=(di == 0), stop=(di == nd - 1))
                    nc.scalar.activation(h[:, hi * BC:(hi + 1) * BC], p1[:],
                                         mybir.ActivationFunctionType.Relu,
                                         bias=bias(hi))
            return h

        def emit_l2(g, e, h, split_eo=False):
            k = _gidx(g) * E + e
            p2 = ps2.tile([128, NB * OUT], F32, name="p2", tag="p2")
            for bi in range(NB):
                dst = p2[:, bi * OUT:(bi + 1) * OUT]
                for hi in range(NH):
                    nc.tensor.matmul(
                        dst,
                        h[:, hi * BC + bi * 128: hi * BC + (bi + 1) * 128],
                        w2s(g, e, hi),
                        start=(hi == 0), stop=False)
                # K=1 ones matmul adds b2 and closes this subtile's group
                nc.tensor.matmul(
                    dst, ones[:],
                    b2t[0:1, (k * NB + bi) * OUT:(k * NB + bi + 1) * OUT],
                    start=False, stop=True)
            eo = eop.tile([128, NB * OUT], BF16, name="eo", tag="eo")
            if split_eo:
                for bi in range(NB):
                    nc.scalar.activation(eo[:, bi * OUT:(bi + 1) * OUT],
                                         p2[:, bi * OUT:(bi + 1) * OUT],
                                         mybir.ActivationFunctionType.Relu)
            else:
                nc.scalar.activation(eo[:], p2[:],
                                     mybir.ActivationFunctionType.Relu)
            return eo

        # expert -> (output index, gate set, gate column) contributions
        # gates: gsh over [t1(0-3), t2(4-7), sh(8-11)]
        #        g1  over [t1(0-3), sh(4-7)]; g2 over [t2(0-3), sh(4-7)]
        def contributions(g, e):
            if g == "t1":
                return [(0, 0, e), (1, 1, e)]
            elif g == "t2":
                return [(0, 0, 4 + e), (2, 2, e)]
            else:
                return [(0, 0, 8 + e), (1, 1, 4 + e), (2, 2, 4 + e)]

        # processing order: t1(0..3), sh(0..3), t2(0..3)
        def _is_last_contrib(g, e, o):
            if o == 1:
                return g == "sh" and e == E - 1
            return g == "t2" and e == E - 1

        gate_ct = [None, None, None]

        def emit_combine(g, e, eo, split_dma=False):
            for bi in range(NB):
                eo_s = eo[:, bi * OUT:(bi + 1) * OUT]
                for (o, gs, col) in contributions(g, e):
                    ng = GATE_NG[gs]
                    g_ap = gate_ct[gs][:, bi * ng + col: bi * ng + col + 1]
                    a = acc[o][:, bi * OUT:(bi + 1) * OUT]
                    if not acc_init[o][bi]:
                        nc.vector.tensor_scalar_mul(a, eo_s, g_ap)
                        acc_init[o][bi] = True
                    else:
                        nc.vector.scalar_tensor_tensor(
                            a, eo_s, g_ap, a,
                            op0=mybir.AluOpType.mult,
                            op1=mybir.AluOpType.add)
                if split_dma and bi == 1:
                    for (o, gs, col) in contributions(g, e):
                        if _is_last_contrib(g, e, o):
                            half = 2 * OUT
                            eng = nc.scalar if o == 2 else nc.sync
                            eng.dma_start(outs[o][:, :half],
                                          acc[o][:, :half])
            for (o, gs, col) in contributions(g, e):
                if _is_last_contrib(g, e, o):
                    if split_dma:
                        half = 2 * OUT
                        eng = nc.scalar if o == 2 else nc.sync
                        eng.dma_start(outs[o][:, half:],
                                      acc[o][:, half:])
                    else:
                        nc.sync.dma_start(outs[o][:], acc[o][:])

        # ---- gates ------------------------------------------------------
        egs = [None, None, None]

        def emit_gate_logits(gi):
            ng = GATE_NG[gi]
            gx = GATE_X[gi]
            nd = GND[gx]
            off = GATE_WOFF[gi]
            lg = ps1.tile([ng, BC], F32, name="lg", tag="p1")
            for di in range(nd):
                nc.tensor.matmul(
                    lg[:], gwt[:, off + di * ng: off + (di + 1) * ng],
                    xs(gx, di),
                    start=(di == 0), stop=(di == nd - 1))
            eg = gp.tile([ng, BC], BF16, name=f"eg{gi}", tag=f"eg{gi}")
            nc.scalar.activation(eg[:], lg[:],
                                 mybir.ActivationFunctionType.Exp,
                                 bias=gbt[0:ng, gi:gi + 1])
            egs[gi] = eg

        def emit_gate_norm(gi):
            ng = GATE_NG[gi]
            eg = egs[gi]
            pt = ps1.tile([128, NB * ng], BF16, name="gtr", tag="p1")
            for bi in range(NB):
                nc.tensor.transpose(pt[:, bi * ng:(bi + 1) * ng],
                                    eg[:, bi * 128:(bi + 1) * 128],
                                    ident[:ng, :ng])
            ct = gp.tile([128, NB * ng], F32, name=f"ct{gi}", tag=f"ct{gi}")
            nc.vector.tensor_copy(ct[:], pt[:])
            st = gp.tile([128, NB], F32, name=f"st{gi}", tag=f"st{gi}")
            rt = gp.tile([128, NB], F32, name=f"rt{gi}", tag=f"rt{gi}")
            for bi in range(NB):
                nc.vector.tensor_reduce(st[:, bi:bi + 1],
                                        ct[:, bi * ng:(bi + 1) * ng],
                                        axis=mybir.AxisListType.X,
                                        op=mybir.AluOpType.add)
            nc.vector.reciprocal(rt[:], st[:])
            for bi in range(NB):
                # pre-scale: combine uses normalized gates directly
                nc.vector.tensor_scalar_mul(ct[:, bi * ng:(bi + 1) * ng],
                                            ct[:, bi * ng:(bi + 1) * ng],
                                            rt[:, bi:bi + 1])
            gate_ct[gi] = ct

        # ---- emission order --------------------------------------------
        # t1_0 (d-outer, earliest PE start), t1_1, g1 logits, t1_2,
        # gsh logits, t1_3, g2 logits, batched gate transposes+normalize
        # (eg ACTs long done by then), deferred t1 combines, then sh/t2.
        eos = {}
        eos[("t1", 0)] = emit_l2("t1", 0, emit_l1("t1", 0, d_outer=True))
        eos[("t1", 1)] = emit_l2("t1", 1, emit_l1("t1", 1))
        emit_gate_logits(1)
        eos[("t1", 2)] = emit_l2("t1", 2, emit_l1("t1", 2))
        emit_gate_logits(0)
        eos[("t1", 3)] = emit_l2("t1", 3, emit_l1("t1", 3))
        emit_gate_logits(2)
        for gi in range(3):
            emit_gate_norm(gi)
        for e in range(E):
            emit_combine("t1", e, eos.pop(("t1", e)))
        for g in ("sh", "t2"):
            for e in range(E):
                last = (g == "t2" and e == E - 1)
                h = emit_l1(g, e)
                eo = emit_l2(g, e, h, split_eo=last)
                emit_combine(g, e, eo, split_dma=last)


_NC_CACHE = None


def _pack_shared(inputs):
    """Host-side packing into SBUF partition layouts + bf16 cast."""
    def pack_w1(w):   # [E, D, HID] -> per-e [128, nd*HID] fp32
        e_, d_, h_ = w.shape
        nd = d_ // 128
        wp = np.asarray(w, np.float32).reshape(e_, nd, 128, h_) \
            .transpose(0, 2, 1, 3).reshape(e_, 128, nd * h_)
        return [wp[e] for e in range(e_)]

    def pack_w2(w):   # [E, HID, OUT] -> [128, E*NH*OUT] fp32
        e_, h_, o_ = w.shape
        nh = h_ // 128
        return np.asarray(w, np.float32).reshape(e_, nh, 128, o_) \
            .transpose(2, 0, 1, 3).reshape(128, e_ * nh * o_)

    def pack_b1(b):   # [E, HID] -> [E, 128, NH]
        e_, h_ = b.shape
        nh = h_ // 128
        return np.asarray(b, np.float32).reshape(e_, nh, 128).transpose(0, 2, 1)

    def pack_gw(w):   # [D, ng] -> [128, nd*ng]
        d_, ng = w.shape
        nd = d_ // 128
        return np.asarray(w, np.float32).reshape(nd, 128, ng) \
            .transpose(1, 0, 2).reshape(128, nd * ng)

    W1 = {"t1": inputs["t1_W1"], "sh": inputs["sh_W1"], "t2": inputs["t2_W1"]}
    W2 = {"t1": inputs["t1_W2"], "sh": inputs["sh_W2"], "t2": inputs["t2_W2"]}
    B1 = {"t1": inputs["t1_b1"], "sh": inputs["sh_b1"], "t2": inputs["t2_b1"]}
    B2 = {"t1": inputs["t1_b2"], "sh": inputs["sh_b2"], "t2": inputs["t2_b2"]}

    # segment content (fp32) by key; x segments filled per core later
    segs = {}
    for g in GROUPS:
        for e, w in enumerate(pack_w1(W1[g])):
            segs[f"w1_{g}_{e}"] = w
        segs[f"w2_{g}"] = pack_w2(W2[g])

    m = {}
    b1 = np.stack([pack_b1(B1[g]) for g in GROUPS])       # [3, E, 128, NH]
    m["b1all"] = np.ascontiguousarray(
        b1.reshape(12, 128, NH).transpose(1, 0, 2).reshape(128, 12 * NH),
        dtype=np.float32)
    b2 = np.stack([np.asarray(B2[g], np.float32) for g in GROUPS])  # [3,E,OUT]
    b2r = np.broadcast_to(b2.reshape(12, 1, OUT), (12, NB, OUT))
    m["b2all"] = np.ascontiguousarray(b2r.reshape(1, 12 * NB * OUT)).astype(BF)

    gw = np.concatenate([pack_gw(inputs["gsh_W"]), pack_gw(inputs["g1_W"]),
                         pack_gw(inputs["g2_W"])], axis=1)  # [128, 160]
    m["gwall"] = np.ascontiguousarray(gw).astype(BF)
    gb = np.zeros((12, 3), np.float32)
    gb[:12, 0] = np.asarray(inputs["gsh_b"], np.float32)
    gb[:8, 1] = np.asarray(inputs["g1_b"], np.float32)
    gb[:8, 2] = np.asarray(inputs["g2_b"], np.float32)
    m["gball"] = gb
    return m, segs


def _pack_xT(x):      # [BC, D] -> [128, nd*BC] fp32
    bc, d_ = x.shape
    nd = d_ // 128
    xt = np.asarray(x, np.float32).T.reshape(nd, 128, bc) \
        .transpose(1, 0, 2).reshape(128, nd * bc)
    return xt


def _build_in_maps(inputs):
    m_const, segs = _pack_shared(inputs)
    xs = {"t1": inputs["x_task1"], "sh": inputs["x_full"],
          "t2": inputs["x_task2"]}

    # bundles without x segments are shared across cores
    shared_bundles = {}
    for bname, parts in BUNDLES + SCALAR_BUNDLES:
        if any(k.startswith("x_") for k, _ in parts):
            continue
        shared_bundles[bname] = np.ascontiguousarray(
            np.concatenate([segs[k] for k, _ in parts], axis=1)).astype(BF)

    in_maps = []
    for c in range(N_CORES):
        rows = slice(c * BC, (c + 1) * BC)
        m = dict(m_const)
        m.update(shared_bundles)
        xseg = {f"x_{g}": _pack_xT(np.asarray(xs[g])[rows]) for g in GROUPS}
        for bname, parts in BUNDLES:
            if bname in shared_bundles:
                continue
            m[bname] = np.ascontiguousarray(np.concatenate(
                [xseg[k] if k.startswith("x_") else segs[k] for k, _ in parts],
                axis=1)).astype(BF)
        in_maps.append(m)
    return in_maps


def _unpack_out(a):   # [128, NB*OUT] -> [BC, OUT]
    return np.ascontiguousarray(
        a.reshape(128, NB, OUT).transpose(1, 0, 2).reshape(BC, OUT))


def kernel(**inputs):
    global _NC_CACHE
    if _NC_CACHE is None:
        _NC_CACHE = _build_nc()
    nc = _NC_CACHE

    in_maps = _build_in_maps(inputs)
    res = run_bass_kernel_spmd(nc, in_maps, list(range(N_CORES)))
    full = []
    for name in ("o_sh", "o_t1", "o_t2"):
        full.append(np.concatenate(
            [_unpack_out(res.results[c][name]) for c in range(N_CORES)]))
    return tuple(full)


# revision 8
# speedup vs baseline: 1.2077x; 1.0297x over previous
"""CGC MoE routing kernel for Trainium2, 8-core data-parallel over batch.

Problem (per reference):
  B=4096, D_FULL=1024, D_T1=D_T2=512, experts: 4 shared (on x_full),
  4 task-1 (on x_task1), 4 task-2 (on x_task2); each expert is a 2-layer
  ReLU MLP (hidden 512, out 256). Three softmax gates combine expert
  outputs into (out_sh, out1, out2), each [4096, 256] fp32.

Strategy: shard the batch 8 ways (512 rows/core), replicate weights.
Each core computes all 12 experts + gates for its shard; host concats.

All matmul operands are cast to bf16 ON THE HOST and packed into the
exact SBUF layout; inputs are bundled into a handful of large HWDGE
DMAs (per-DMA fixed cost ~1.5-2us, so few big transfers beat many
small ones) ordered by first use:
  sync ring:   x1 | w1_t1_{0,1,2} | w1_t1_3+xf+x2 | w1_sh | w1_t2
  scalar ring: consts(gw,gb,b1,b2) | w2_t1 | w2_sh | w2_t2
Layers:
  L1: H[h,b]  = sum_d W1[d,h].T @ X'[d,b]   (per-hi PSUM bank, ACT ReLU
      with per-partition b1 bias)
  L2: EO[b,o] = sum_h H[h,b].T  @ W2[h,o] accumulated into one
      [128, 4*256] PSUM tile; one ones-row K=1 matmul adds b2 for all
      4 b-subtiles; ONE batched ACT ReLU -> bf16 eo (the last expert
      instead splits eo per b-subtile so its combine pipeline drains
      early).
  Gates: logits[g,b] on PE; exp via ACT with bias; 4 PE-transposes into
      one PSUM tile; batched copy + rowsum/recip/scale -> pre-normalized
      gate columns. Gate sets are emitted interleaved between t1
      experts (g1 after t1_1, gsh after t1_2, g2 after t1_3).
  Combine: acc[b,o] (+)= EO_e[b,o] * gate_col[b,1] on DVE
      (scalar_tensor_tensor); acc is one [128, 4*256] fp32 tile per
      output; one output DMA per output (split in halves for the two
      last-finishing outputs); host unpacks [128,4,256].
"""
import sys
import numpy as np

sys.path.insert(0, "/opt/trn_rl_repo")

import ml_dtypes

import concourse.bass as bass
import concourse.mybir as mybir
import concourse.tile as tile
import concourse.masks as masks
from concourse.bass_utils import run_bass_kernel_spmd

F32 = mybir.dt.float32
BF16 = mybir.dt.bfloat16
BF = ml_dtypes.bfloat16

B = 4096
N_CORES = 8
BC = B // N_CORES          # 512 rows per core
DF, D1, D2 = 1024, 512, 512
HID, OUT = 512, 256
NB = BC // 128             # 4 batch tiles per core
NH = HID // 128            # 4 hidden tiles
E = 4                      # experts per group
GROUPS = ["t1", "sh", "t2"]           # processing order
GND = {"t1": D1 // 128, "sh": DF // 128, "t2": D2 // 128}
# gate sets: 0=gsh(x_full,12), 1=g1(x_task1,8), 2=g2(x_task2,8)
GATE_NG = [12, 8, 8]
GATE_X = ["sh", "t1", "t2"]           # which x feeds each gate
GATE_WOFF = [0, 96, 128]              # col offset in packed gate weights

# ---- bundled DMA layout (all bf16, [128, width] each) ----------------------
# widths in bf16 elements per partition
W_X = {g: GND[g] * BC for g in GROUPS}
W_W1 = {g: GND[g] * HID for g in GROUPS}
W_W2 = E * NH * OUT
BUNDLES = [
    # startup quarters: x1 d-tile + matching first-expert W1 d-tile per DMA
    # so the d-outer first expert consumes arrivals progressively
    ("bun_s0", [("x_t1_d0", BC), ("w1_t1_0_d0", HID)]),
    ("bun_s1", [("x_t1_d1", BC), ("w1_t1_0_d1", HID)]),
    ("bun_s2", [("x_t1_d2", BC), ("w1_t1_0_d2", HID)]),
    ("bun_s3", [("x_t1_d3", BC), ("w1_t1_0_d3", HID)]),
    ("bun_w1a", [("w1_t1_1", W_W1["t1"]), ("w1_t1_2", W_W1["t1"])]),
    ("bun_w1b", [("w1_t1_3", W_W1["t1"]), ("x_sh", W_X["sh"]),
                 ("x_t2", W_X["t2"])]),
    ("bun_w1sh", [(f"w1_sh_{e}", W_W1["sh"]) for e in range(E)]),
    ("bun_w1t2", [(f"w1_t2_{e}", W_W1["t2"]) for e in range(E)]),
]
SCALAR_BUNDLES = [
    ("bun_w2t1", [("w2_t1", W_W2)]),
    ("bun_w2sh", [("w2_sh", W_W2)]),
    ("bun_w2t2", [("w2_t2", W_W2)]),
]


def _gidx(group):
    return GROUPS.index(group)


def _legalize_waits(nc, max_waits: int = 1):
    """This walrus build supports a single sync wait per instruction;
    hoist extra waits onto standalone single-wait EventSemaphore
    instructions inserted just before (same engine, same order)."""
    uid = 0
    for f in nc.m.functions:
        for blk in f.blocks:
            out = []
            changed = False
            for inst in blk.instructions:
                si = inst.sync_info
                ow = list(si.on_wait) if si and si.on_wait else []
                if len(ow) > max_waits:
                    changed = True
                    for w in ow[:-max_waits]:
                        ev = mybir.InstEventSemaphore(
                            name=f"legalw-{uid}",
                            sync_info=mybir.SyncInfo(on_wait=[w], on_update=[]),
                        )
                        uid += 1
                        ev.engine = inst.engine
                        out.append(ev)
                    inst.sync_info = mybir.SyncInfo(
                        on_wait=ow[-max_waits:],
                        on_update=list(si.on_update) if si.on_update else [],
                    )
                out.append(inst)
            if changed:
                blk.instructions = out
    return nc


def _build_nc():
    nc = bass.Bass()

    def din(name, shape, dt):
        return nc.declare_dram_parameter(name, list(shape), dt, isOutput=False)

    bun_in = {}
    for bname, parts in BUNDLES + SCALAR_BUNDLES:
        w = sum(p[1] for p in parts)
        bun_in[bname] = din(bname, (128, w), BF16)
    b1in = din("b1all", (128, 12 * NH), F32)        # [:, k*NH+hi]
    b2in = din("b2all", (1, 12 * NB * OUT), BF16)   # b2 repeated per btile
    gwin = din("gwall", (128, 160), BF16)
    gbin = din("gball", (12, 3), F32)

    outs = [nc.declare_dram_parameter(n, [128, NB * OUT], F32, isOutput=True)
            for n in ("o_sh", "o_t1", "o_t2")]

    with tile.TileContext(nc) as tc:
        _emit(nc, tc, bun_in, b1in, b2in, gwin, gbin, outs)
    _legalize_waits(nc)
    return nc


def _emit(nc, tc, bun_in, b1in, b2in, gwin, gbin, outs):
    from contextlib import ExitStack
    ctx = ExitStack()
    with ctx:
        wp = ctx.enter_context(tc.tile_pool(name="wp", bufs=1))
        cp = ctx.enter_context(tc.tile_pool(name="cp", bufs=1))
        hp = ctx.enter_context(tc.tile_pool(name="hp", bufs=2))
        eop = ctx.enter_context(tc.tile_pool(name="eop", bufs=6))
        accp = ctx.enter_context(tc.tile_pool(name="accp", bufs=1))
        gp = ctx.enter_context(tc.tile_pool(name="gp", bufs=1))
        ps1 = ctx.enter_context(tc.tile_pool(name="ps1", bufs=4, space="PSUM"))
        ps2 = ctx.enter_context(tc.tile_pool(name="ps2", bufs=2, space="PSUM"))

        # ---- input DMAs (HWDGE), bundled, ordered by first use ----------
        # seg maps logical key -> (tile, col offset, width)
        seg = {}

        def load_bundle(bname, parts, engine):
            w = sum(p[1] for p in parts)
            t = wp.tile([128, w], BF16, name=bname, tag=bname)
            engine.dma_start(t[:], bun_in[bname][:])
            off = 0
            for key, width in parts:
                seg[key] = (t, off, width)
                off += width

        # interleave: consts first on scalar ring (tiny), then the big
        # sync-ring stream; W2 groups stream on the scalar ring.
        for bi_ in range(4):                        # startup quarters
            load_bundle(*BUNDLES[bi_], nc.sync)
        load_bundle(*SCALAR_BUNDLES[0], nc.scalar)  # w2_t1 (needed ~12us)
        gwt = cp.tile([128, 160], BF16, tag="gw")
        nc.scalar.dma_start(gwt[:], gwin[:])
        gbt = cp.tile([12, 3], F32, tag="gb")
        nc.scalar.dma_start(gbt[:], gbin[:])
        b1t = cp.tile([128, 12 * NH], F32, tag="b1")
        nc.scalar.dma_start(b1t[:], b1in[:])
        b2t = cp.tile([1, 12 * NB * OUT], BF16, tag="b2")
        nc.scalar.dma_start(b2t[:], b2in[:])
        for bname, parts in BUNDLES[4:]:
            load_bundle(bname, parts, nc.sync)
        for bname, parts in SCALAR_BUNDLES[1:]:
            load_bundle(bname, parts, nc.scalar)

        def xs(g, di):
            if f"x_{g}_d{di}" in seg:
                t, off, _ = seg[f"x_{g}_d{di}"]
                return t[:, off: off + BC]
            t, off, _ = seg[f"x_{g}"]
            return t[:, off + di * BC: off + (di + 1) * BC]

        def w1s(g, e, di, hi):
            if f"w1_{g}_{e}_d{di}" in seg:
                t, off, _ = seg[f"w1_{g}_{e}_d{di}"]
                return t[:, off + hi * 128: off + (hi + 1) * 128]
            t, off, _ = seg[f"w1_{g}_{e}"]
            o = off + di * HID + hi * 128
            return t[:, o: o + 128]

        def w2s(g, e, hi):
            t, off, _ = seg[f"w2_{g}"]
            o = off + (e * NH + hi) * OUT
            return t[:, o: o + OUT]

        # identity for PE transpose; ones row for K=1 bias matmuls
        # (memset/iota are invalid ISA at bf16 -> build fp32, cast-copy)
        ident32 = cp.tile([128, 128], F32, tag="ident32")
        masks.make_identity(nc, ident32[:])
        ident = cp.tile([128, 128], BF16, tag="ident")
        nc.vector.tensor_copy(ident[:], ident32[:])
        ones32 = cp.tile([1, 128], F32, tag="ones32")
        nc.vector.memset(ones32[:], 1.0)
        ones = cp.tile([1, 128], BF16, tag="ones")
        nc.vector.tensor_copy(ones[:], ones32[:])

        # accumulator tiles [128, NB*OUT] fp32 per output
        acc = [accp.tile([128, NB * OUT], F32, name=f"acc{o}", tag=f"acc{o}")
               for o in range(3)]
        acc_init = [[False] * NB for _ in range(3)]

        # ---- expert bodies ---------------------------------------------
        def emit_l1(g, e, d_outer=False):
            nd = GND[g]
            k = _gidx(g) * E + e
            h = hp.tile([128, NH * BC], BF16, name="h", tag="h")

            def bias(hi):
                return b1t[:, k * NH + hi: k * NH + hi + 1]

            if d_outer:
                ps = [ps1.tile([128, BC], F32, name=f"p1_{hi}", tag="p1")
                      for hi in range(NH)]
                for di in range(nd - 1):
                    for hi in range(NH):
                        nc.tensor.matmul(
                            ps[hi][:], w1s(g, e, di, hi), xs(g, di),
                            start=(di == 0), stop=False)
                for hi in range(NH):
                    # close each hi on the last d-tile, ACT right away so
                    # ReLUs overlap the remaining hi matmuls
                    nc.tensor.matmul(
                        ps[hi][:], w1s(g, e, nd - 1, hi), xs(g, nd - 1),
                        start=False, stop=True)
                    nc.scalar.activation(h[:, hi * BC:(hi + 1) * BC],
                                         ps[hi][:],
                                         mybir.ActivationFunctionType.Relu,
                                         bias=bias(hi))
            else:
                for hi in range(NH):
                    p1 = ps1.tile([128, BC], F32, name="p1", tag="p1")
                    for di in range(nd):
                        nc.tensor.matmul(
                            p1[:], w1s(g, e, di, hi), xs(g, di),
                            start=(di == 0), stop=(di == nd - 1))
                    nc.scalar.activation(h[:, hi * BC:(hi + 1) * BC], p1[:],
                                         mybir.ActivationFunctionType.Relu,
                                         bias=bias(hi))
            return h

        def emit_l2(g, e, h, split_eo=False):
            k = _gidx(g) * E + e
            p2 = ps2.tile([128, NB * OUT], F32, name="p2", tag="p2")
            for bi in range(NB):
                dst = p2[:, bi * OUT:(bi + 1) * OUT]
                for hi in range(NH):
                    nc.tensor.matmul(
                        dst,
                        h[:, hi * BC + bi * 128: hi * BC + (bi + 1) * 128],
                        w2s(g, e, hi),
                        start=(hi == 0), stop=False)
                # K=1 ones matmul adds b2 and closes this subtile's group
                nc.tensor.matmul(
                    dst, ones[:],
                    b2t[0:1, (k * NB + bi) * OUT:(k * NB + bi + 1) * OUT],
                    start=False, stop=True)
            eo = eop.tile([128, NB * OUT], BF16, name="eo", tag="eo")
            if split_eo:
                for bi in range(NB):
                    nc.scalar.activation(eo[:, bi * OUT:(bi + 1) * OUT],
                                         p2[:, bi * OUT:(bi + 1) * OUT],
                                         mybir.ActivationFunctionType.Relu)
            else:
                nc.scalar.activation(eo[:], p2[:],
                                     mybir.ActivationFunctionType.Relu)
            return eo

        # expert -> (output index, gate set, gate column) contributions
        # gates: gsh over [t1(0-3), t2(4-7), sh(8-11)]
        #        g1  over [t1(0-3), sh(4-7)]; g2 over [t2(0-3), sh(4-7)]
        def contributions(g, e):
            if g == "t1":
                return [(0, 0, e), (1, 1, e)]
            elif g == "t2":
                return [(0, 0, 4 + e), (2, 2, e)]
            else:
                return [(0, 0, 8 + e), (1, 1, 4 + e), (2, 2, 4 + e)]

        # processing order: t1(0..3), sh(0..3), t2(0..3)
        def _is_last_contrib(g, e, o):
            if o == 1:
                return g == "sh" and e == E - 1
            return g == "t2" and e == E - 1

        gate_ct = [None, None, None]

        def emit_combine(g, e, eo, split_dma=False):
            for bi in range(NB):
                eo_s = eo[:, bi * OUT:(bi + 1) * OUT]
                for (o, gs, col) in contributions(g, e):
                    ng = GATE_NG[gs]
                    g_ap = gate_ct[gs][:, bi * ng + col: bi * ng + col + 1]
                    a = acc[o][:, bi * OUT:(bi + 1) * OUT]
                    if not acc_init[o][bi]:
                        nc.vector.tensor_scalar_mul(a, eo_s, g_ap)
                        acc_init[o][bi] = True
                    else:
                        nc.vector.scalar_tensor_tensor(
                            a, eo_s, g_ap, a,
                            op0=mybir.AluOpType.mult,
                            op1=mybir.AluOpType.add)
                if split_dma and bi == 1:
                    for (o, gs, col) in contributions(g, e):
                        if _is_last_contrib(g, e, o):
                            half = 2 * OUT
                            eng = nc.scalar if o == 2 else nc.sync
                            eng.dma_start(outs[o][:, :half],
                                          acc[o][:, :half])
            for (o, gs, col) in contributions(g, e):
                if _is_last_contrib(g, e, o):
                    if split_dma:
                        half = 2 * OUT
                        eng = nc.scalar if o == 2 else nc.sync
                        eng.dma_start(outs[o][:, half:],
                                      acc[o][:, half:])
                    else:
                        nc.sync.dma_start(outs[o][:], acc[o][:])

        # ---- gates ------------------------------------------------------
        egs = [None, None, None]

        def emit_gate_logits(gi):
            ng = GATE_NG[gi]
            gx = GATE_X[gi]
            nd = GND[gx]
            off = GATE_WOFF[gi]
            lg = ps1.tile([ng, BC], F32, name="lg", tag="p1")
            for di in range(nd):
                nc.tensor.matmul(
                    lg[:], gwt[:, off + di * ng: off + (di + 1) * ng],
                    xs(gx, di),
                    start=(di == 0), stop=(di == nd - 1))
            eg = gp.tile([ng, BC], BF16, name=f"eg{gi}", tag=f"eg{gi}")
            nc.scalar.activation(eg[:], lg[:],
                                 mybir.ActivationFunctionType.Exp,
                                 bias=gbt[0:ng, gi:gi + 1])
            egs[gi] = eg

        def emit_gate_norm(gi):
            ng = GATE_NG[gi]
            eg = egs[gi]
            pt = ps1.tile([128, NB * ng], BF16, name="gtr", tag="p1")
            for bi in range(NB):
                nc.tensor.transpose(pt[:, bi * ng:(bi + 1) * ng],
                                    eg[:, bi * 128:(bi + 1) * 128],
                                    ident[:ng, :ng])
            ct = gp.tile([128, NB * ng], F32, name=f"ct{gi}", tag=f"ct{gi}")
            nc.vector.tensor_copy(ct[:], pt[:])
            st = gp.tile([128, NB], F32, name=f"st{gi}", tag=f"st{gi}")
            rt = gp.tile([128, NB], F32, name=f"rt{gi}", tag=f"rt{gi}")
            for bi in range(NB):
                nc.vector.tensor_reduce(st[:, bi:bi + 1],
                                        ct[:, bi * ng:(bi + 1) * ng],
                                        axis=mybir.AxisListType.X,
                                        op=mybir.AluOpType.add)
            nc.vector.reciprocal(rt[:], st[:])
            for bi in range(NB):
                # pre-scale: combine uses normalized gates directly
                nc.vector.tensor_scalar_mul(ct[:, bi * ng:(bi + 1) * ng],
                                            ct[:, bi * ng:(bi + 1) * ng],
                                            rt[:, bi:bi + 1])
            gate_ct[gi] = ct

        # ---- emission order --------------------------------------------
        # t1_0 (d-outer, earliest PE start), t1_1, g1 logits, t1_2,
        # gsh logits, t1_3, g2 logits, batched gate transposes+normalize
        # (eg ACTs long done by then), deferred t1 combines, then sh/t2.
        eos = {}
        emit_gate_logits(1)       # needs only x1 quarters + gw: PE warmup
        eos[("t1", 0)] = emit_l2("t1", 0, emit_l1("t1", 0, d_outer=True))
        eos[("t1", 1)] = emit_l2("t1", 1, emit_l1("t1", 1))
        eos[("t1", 2)] = emit_l2("t1", 2, emit_l1("t1", 2))
        emit_gate_logits(0)
        eos[("t1", 3)] = emit_l2("t1", 3, emit_l1("t1", 3))
        emit_gate_logits(2)
        emit_gate_norm(1)
        emit_gate_norm(0)
        for e in range(E):
            emit_combine("t1", e, eos.pop(("t1", e)))
        first_sh = emit_l2("sh", 0, emit_l1("sh", 0))
        emit_gate_norm(2)
        emit_combine("sh", 0, first_sh)
        for g in ("sh", "t2"):
            for e in range(E):
                if g == "sh" and e == 0:
                    continue
                last = (g == "t2" and e == E - 1)
                h = emit_l1(g, e)
                eo = emit_l2(g, e, h, split_eo=last)
                emit_combine(g, e, eo, split_dma=last)


_NC_CACHE = None


def _pack_shared(inputs):
    """Host-side packing into SBUF partition layouts + bf16 cast."""
    def pack_w1(w):   # [E, D, HID] -> per-e [128, nd*HID] fp32
        e_, d_, h_ = w.shape
        nd = d_ // 128
        wp = np.asarray(w, np.float32).reshape(e_, nd, 128, h_) \
            .transpose(0, 2, 1, 3).reshape(e_, 128, nd * h_)
        return [wp[e] for e in range(e_)]

    def pack_w2(w):   # [E, HID, OUT] -> [128, E*NH*OUT] fp32
        e_, h_, o_ = w.shape
        nh = h_ // 128
        return np.asarray(w, np.float32).reshape(e_, nh, 128, o_) \
            .transpose(2, 0, 1, 3).reshape(128, e_ * nh * o_)

    def pack_b1(b):   # [E, HID] -> [E, 128, NH]
        e_, h_ = b.shape
        nh = h_ // 128
        return np.asarray(b, np.float32).reshape(e_, nh, 128).transpose(0, 2, 1)

    def pack_gw(w):   # [D, ng] -> [128, nd*ng]
        d_, ng = w.shape
        nd = d_ // 128
        return np.asarray(w, np.float32).reshape(nd, 128, ng) \
            .transpose(1, 0, 2).reshape(128, nd * ng)

    W1 = {"t1": inputs["t1_W1"], "sh": inputs["sh_W1"], "t2": inputs["t2_W1"]}
    W2 = {"t1": inputs["t1_W2"], "sh": inputs["sh_W2"], "t2": inputs["t2_W2"]}
    B1 = {"t1": inputs["t1_b1"], "sh": inputs["sh_b1"], "t2": inputs["t2_b1"]}
    B2 = {"t1": inputs["t1_b2"], "sh": inputs["sh_b2"], "t2": inputs["t2_b2"]}

    # segment content (fp32) by key; x segments filled per core later
    segs = {}
    for g in GROUPS:
        for e, w in enumerate(pack_w1(W1[g])):
            segs[f"w1_{g}_{e}"] = w
        segs[f"w2_{g}"] = pack_w2(W2[g])

    m = {}
    b1 = np.stack([pack_b1(B1[g]) for g in GROUPS])       # [3, E, 128, NH]
    m["b1all"] = np.ascontiguousarray(
        b1.reshape(12, 128, NH).transpose(1, 0, 2).reshape(128, 12 * NH),
        dtype=np.float32)
    b2 = np.stack([np.asarray(B2[g], np.float32) for g in GROUPS])  # [3,E,OUT]
    b2r = np.broadcast_to(b2.reshape(12, 1, OUT), (12, NB, OUT))
    m["b2all"] = np.ascontiguousarray(b2r.reshape(1, 12 * NB * OUT)).astype(BF)

    gw = np.concatenate([pack_gw(inputs["gsh_W"]), pack_gw(inputs["g1_W"]),
                         pack_gw(inputs["g2_W"])], axis=1)  # [128, 160]
    m["gwall"] = np.ascontiguousarray(gw).astype(BF)
    gb = np.zeros((12, 3), np.float32)
    gb[:12, 0] = np.asarray(inputs["gsh_b"], np.float32)
    gb[:8, 1] = np.asarray(inputs["g1_b"], np.float32)
    gb[:8, 2] = np.asarray(inputs["g2_b"], np.float32)
    m["gball"] = gb
    return m, segs


def _pack_xT(x):      # [BC, D] -> [128, nd*BC] fp32
    bc, d_ = x.shape
    nd = d_ // 128
    xt = np.asarray(x, np.float32).T.reshape(nd, 128, bc) \
        .transpose(1, 0, 2).reshape(128, nd * bc)
    return xt


def _build_in_maps(inputs):
    m_const, segs = _pack_shared(inputs)
    xs = {"t1": inputs["x_task1"], "sh": inputs["x_full"],
          "t2": inputs["x_task2"]}

    # split first-expert W1 / x into per-d segments for the startup quarters
    w10 = segs["w1_t1_t0_split"] if False else segs["w1_t1_0"]
    for di in range(GND["t1"]):
        segs[f"w1_t1_0_d{di}"] = w10[:, di * HID:(di + 1) * HID]

    # bundles without x segments are shared across cores
    shared_bundles = {}
    for bname, parts in BUNDLES + SCALAR_BUNDLES:
        if any(k.startswith("x_") for k, _ in parts):
            continue
        shared_bundles[bname] = np.ascontiguousarray(
            np.concatenate([segs[k] for k, _ in parts], axis=1)).astype(BF)

    in_maps = []
    for c in range(N_CORES):
        rows = slice(c * BC, (c + 1) * BC)
        m = dict(m_const)
        m.update(shared_bundles)
        xseg = {}
        for g in GROUPS:
            xp = _pack_xT(np.asarray(xs[g])[rows])
            xseg[f"x_{g}"] = xp
            if g == "t1":
                for di in range(GND[g]):
                    xseg[f"x_{g}_d{di}"] = xp[:, di * BC:(di + 1) * BC]
        for bname, parts in BUNDLES:
            if bname in shared_bundles:
                continue
            m[bname] = np.ascontiguousarray(np.concatenate(
                [xseg[k] if k.startswith("x_") else segs[k] for k, _ in parts],
                axis=1)).astype(BF)
        in_maps.append(m)
    return in_maps


def _unpack_out(a):   # [128, NB*OUT] -> [BC, OUT]
    return np.ascontiguousarray(
        a.reshape(128, NB, OUT).transpose(1, 0, 2).reshape(BC, OUT))


def kernel(**inputs):
    global _NC_CACHE
    if _NC_CACHE is None:
        _NC_CACHE = _build_nc()
    nc = _NC_CACHE

    in_maps = _build_in_maps(inputs)
    res = run_bass_kernel_spmd(nc, in_maps, list(range(N_CORES)))
    full = []
    for name in ("o_sh", "o_t1", "o_t2"):
        full.append(np.concatenate(
            [_unpack_out(res.results[c][name]) for c in range(N_CORES)]))
    return tuple(full)
